# revision 1
# baseline (speedup 1.0000x reference)
"""Multi-head attention (B=2, S=2048, H=8, Dh=32, D=256) on 8 TRN2 NeuronCores.

Sharding: core c -> (batch b = c//4, query-block qb = c%4 of 512 rows).
Each core computes full attention + output projection for its 512 query rows;
no cross-core communication is needed.  Host does layout prep only
(transposes + bf16 casts); all FLOPs run on device.

Device-side layout (per core):
  - raw activations shipped transposed: qT [256f, 512q], kT/vT [256f, 2048s]
  - q/k projections produced directly transposed (head-dim on partitions,
    head h at partitions 32*(h%4) of free-block h//4) so QK^T runs as
    K=32 row-tiled matmuls (4 concurrent heads via tile_position).
  - scores computed TRANSPOSED: scoresT[k, q]; exp is a straight ScalarE
    pass over PSUM (no max subtraction: scores ~ N(0,1)); ScalarE is the
    bottleneck engine (~8.4M exps/core), everything else hides behind it.
  - v projected to natural layout augmented with a ones column per head
    ([128, 16, 8, 33]) so each PV matmul (M=33) also accumulates the
    softmax denominator as its last output row - no separate reduction.
  - normalization: reciprocal of the two denominator rows (partitions 32
    and 96) + DMA row-broadcast to partitions 0-31/64-95 + one full-width
    DVE multiply per PV accumulator.
  - final projection: K=32 matmuls per head slice against a host-permuted
    WoT whose row bands match the PV output partition bases.
"""

import sys

sys.path.insert(0, "/opt/trn_rl_repo")

import numpy as np
import ml_dtypes

import concourse.bass as bass
import concourse.bacc as bacc
import concourse.mybir as mybir
from concourse.tile import TileContext
from concourse.bass import ts
from concourse.bass_utils import run_bass_kernel_spmd

BF16 = mybir.dt.bfloat16
F32 = mybir.dt.float32
EXP = mybir.ActivationFunctionType.Exp

B, SEQ, D = 2, 2048, 256
H, DH = 8, 32
QB = 512  # query rows per core
NKT = SEQ // 128  # 16 k-chunk tiles (partition tiles of scoresT)
GROUPS = [(2 * i, 2) for i in range(8)]


def _build_graph():
    nc = bacc.Bacc("TRN2", target_bir_lowering=False, debug=False)

    qT = nc.declare_dram_parameter("qT", [D, QB], BF16, isOutput=False)
    kT = nc.declare_dram_parameter("kT", [D, SEQ], BF16, isOutput=False)
    vT = nc.declare_dram_parameter("vT", [D, SEQ], BF16, isOutput=False)
    wqT = nc.declare_dram_parameter("wqT", [D, D], BF16, isOutput=False)
    wkT = nc.declare_dram_parameter("wkT", [D, D], BF16, isOutput=False)
    wvT = nc.declare_dram_parameter("wvT", [D + 1, H * (DH + 1)], BF16, isOutput=False)
    woP = nc.declare_dram_parameter("woP", [128, 4 * D], BF16, isOutput=False)
    bo = nc.declare_dram_parameter("bo", [D, 1], F32, isOutput=False)
    outT = nc.declare_dram_parameter("outT", [D, QB], F32, isOutput=True)

    with TileContext(nc) as tc:
        with (
            tc.tile_pool(name="cst", bufs=1) as cst,
            tc.tile_pool(name="sb", bufs=1) as sb,
            tc.tile_pool(name="ps", bufs=2, space="PSUM") as ps,
        ):
            # warm loads the exp table set early so the ~2.7us
            # ACT_TABLE_LOAD overlaps the DMA/projection phase.
            warm = cst.tile([1, 1], F32)
            nc.vector.memset(warm[:], 0.0)
            nc.scalar.activation(warm[:], warm[:], EXP)

            # ---- inputs.  k-path first (it gates the first exp); bulk of
            # the v-path goes on the gpsimd DMA queue in parallel.
            wk_t = cst.tile([128, 2, D], BF16)
            wq_t = cst.tile([128, 2, D], BF16)
            kT_t = cst.tile([128, 2, SEQ], BF16)
            qT_t = cst.tile([128, 2, QB], BF16)
            wv_t = cst.tile([128, 2, H * (DH + 1)], BF16)
            wva_t = cst.tile([1, H * (DH + 1)], BF16)
            vT_t = cst.tile([128, 2, SEQ], BF16)
            wo_t = cst.tile([128, 4, D], BF16)
            bo_t = cst.tile([128, 2, 1], F32)
            # k-path on the sync HWDGE queue (s4=0 chunks first - they gate
            # the first QK group); q-path on the scalar HWDGE queue, which is
            # idle until the first exp.
            for f in range(2):
                nc.sync.dma_start(wk_t[:, f, :], wkT[ts(f, 128), :])
            for f in range(2):
                nc.scalar.dma_start(wq_t[:, f, :], wqT[ts(f, 128), :])
                nc.scalar.dma_start(qT_t[:, f, :], qT[ts(f, 128), :])
            for s4 in range(4):
                for f in range(2):
                    nc.sync.dma_start(
                        kT_t[:, f, ts(s4, 512)], kT[ts(f, 128), ts(s4, 512)]
                    )
            for f in range(2):
                nc.scalar.dma_start(wv_t[:, f, :], wvT[ts(f, 128), :])
                nc.scalar.dma_start(vT_t[:, f, :], vT[ts(f, 128), :])
            nc.scalar.dma_start(wva_t[:], wvT[D : D + 1, :])
            nc.scalar.dma_start(wo_t[:, :, :], woP.rearrange("p (b d) -> p b d", b=4))
            for o in range(2):
                nc.scalar.dma_start(bo_t[:, o, :], bo[ts(o, 128), :])

            # ---- projections.  q/k transposed, v natural + ones column.
            qp = cst.tile([128, 2, QB], BF16)  # q_projT
            kp = cst.tile([128, 2, SEQ], BF16)  # k_projT
            vp = cst.tile([128, NKT, H * (DH + 1)], BF16)  # v_proj + ones cols
            ones_t = cst.tile([128, 64], BF16)
            nc.vector.memset(ones_t[:], 1.0)
            vrow1 = cst.tile([1, SEQ], BF16)
            nc.vector.memset(vrow1[:], 1.0)

            def proj_k(m, chunks=(0, 1, 2, 3)):
                for s4 in chunks:
                    pk = ps.tile([128, 512], F32, tag="po", bufs=2, name=f"pk{m}{s4}")
                    for f in range(2):
                        nc.tensor.matmul(
                            pk[:],
                            wk_t[:, f, ts(m, 128)],
                            kT_t[:, f, ts(s4, 512)],
                            start=(f == 0),
                            stop=(f == 1),
                        )
                    nc.vector.tensor_copy(kp[:, m, ts(s4, 512)], pk[:])

            def proj_q(m):
                pq = ps.tile([128, QB], F32, tag="po", bufs=2, name=f"pq{m}")
                for f in range(2):
                    nc.tensor.matmul(
                        pq[:],
                        wq_t[:, f, ts(m, 128)],
                        qT_t[:, f, :],
                        start=(f == 0),
                        stop=(f == 1),
                    )
                nc.vector.tensor_copy(qp[:, m, :], pq[:])

            def proj_v(st):
                # third K=1 matmul of the host-side ones row against the
                # augmented Wv row produces the per-head ones columns, so
                # the PSUM->SBUF copy is fully contiguous (strided DVE
                # writes misbehave on HW).
                pv = ps.tile([128, H * (DH + 1)], F32, tag="po", bufs=2, name=f"pv{st}")
                for f in range(2):
                    nc.tensor.matmul(
                        pv[:],
                        vT_t[:, f, ts(st, 128)],
                        wv_t[:, f, :],
                        start=(f == 0),
                        stop=False,
                    )
                nc.tensor.matmul(
                    pv[:],
                    vrow1[0:1, ts(st, 128)],
                    wva_t[:],
                    start=False,
                    stop=True,
                )
                nc.vector.tensor_copy(vp[:, st, :], pv[:])

            # attn[(m, r)] = exp(scoresT) for head 4m+r: [k-chunk part, q]
            attn = {}
            for m in range(2):
                for r in range(4):
                    attn[(m, r)] = sb.tile(
                        [128, NKT, 512], BF16, tag="attn", bufs=8,
                        name=f"attn{m}{r}",
                    )

            def stage_a_qk(m, pair, g, scs_by_g):
                """QK matmuls for group g of head-pair `pair` in quad m."""
                c0, gsz = GROUPS[g]
                scs = {}
                for r in pair:
                    scs[r] = ps.tile(
                        [128, gsz, 512], F32, tag="sc", bufs=3, name=f"sc{m}{g}{r}"
                    )
                scs_by_g[g] = scs
                for cc in range(gsz):
                    ct = c0 + cc
                    for r in pair:
                        nc.tensor.matmul(
                            scs[r][:, cc, :],
                            kp[ts(r, 32), m, ts(ct, 128)],
                            qp[ts(r, 32), m, :],
                            start=True,
                            stop=True,
                            tile_position=(32 * r, 0),
                        )

            def stage_a_act(m, pair, g, scs_by_g):
                c0, gsz = GROUPS[g]
                for r in pair:
                    nc.scalar.activation(
                        attn[(m, r)][:, c0 : c0 + gsz, :], scs_by_g[g][r][:], EXP
                    )

            def stage_b_chunks(m, po_ab, cts, rs=(0, 1, 2, 3)):
                """PV (M=33, with fused denominator row) for k-chunks cts."""
                for ct in cts:
                    for r in rs:
                        po = po_ab[r // 2]
                        base = 64 * (r % 2)
                        nc.tensor.matmul(
                            po[base : base + DH + 1, :],
                            vp[:, ct, ts(4 * m + r, DH + 1)],
                            attn[(m, r)][:, ct, :],
                            start=(ct == 0),
                            stop=(ct == NKT - 1),
                            tile_position=(0, base),
                            skip_group_check=True,
                        )

            prod = {}

            def stage_c_tile(m, t, po):
                """normalize: prod = po * (1 / PE-broadcast(denominator rows)).

                The denominator rows sit at partitions 32/96 of each PV
                accumulator; a K=1 matmul against a ones sliver replicates
                each across its head's 32 output partitions (DVE/ACT cannot
                move data across partitions).  All reads/writes stay inside
                regions this kernel wrote (PSUM slots are recycled - stale
                bytes belong to other live tiles)."""
                dsb = sb.tile(
                    [128, 512], BF16, tag="dsb", bufs=2, name=f"dsb{m}{t}"
                )
                bc = ps.tile([128, 512], F32, tag="sc", bufs=3, name=f"bc{m}{t}")
                rsb = sb.tile([128, 512], F32, tag="rsb", bufs=2, name=f"rsb{m}{t}")
                prod[(m, t)] = sb.tile(
                    [128, 512], BF16, tag="prod", bufs=4, name=f"prod{m}{t}"
                )
                for base in (0, 64):
                    nc.vector.tensor_copy(
                        dsb[base + DH : base + DH + 1, :],
                        po[base + DH : base + DH + 1, :],
                    )
                    # M=64 fills bc completely so the full-tile reciprocal
                    # below reads no stale slot bytes (the custom DVE
                    # reciprocal only works on full 128-partition tiles).
                    nc.tensor.matmul(
                        bc[base : base + 64, :],
                        ones_t[base + DH : base + DH + 1, :],
                        dsb[base + DH : base + DH + 1, :],
                        start=True,
                        stop=True,
                        tile_position=(base + DH, base),
                        skip_group_check=True,
                    )
                nc.vector.reciprocal_approx_fast(rsb[:], bc[:])
                for base in (0, 64):
                    nc.vector.tensor_mul(
                        prod[(m, t)][base : base + DH, :],
                        po[base : base + DH, :],
                        rsb[base : base + DH, :],
                    )

            # ================= schedule =================
            # minimal prefix to start exps (~QK group g needs kp chunks
            # 3g..3g+2, so chunk s4 of the k-projection is threaded into
            # the loop one group ahead of its consumer)
            proj_k(0, (0,))
            proj_q(0)

            # quad 0 scores/exp; m=1 projections + v projections ride the
            # PE slack while ScalarE chews on quad-0 exps.  QK for group g+1
            # is emitted before exp of group g so ScalarE always has a
            # filled PSUM group waiting.
            sg = {}
            stage_a_qk(0, (0, 1), 0, sg)
            for g in range(8):
                if g in (0, 2, 4):
                    proj_k(0, (g // 2 + 1,))
                if g < 7:
                    stage_a_qk(0, (0, 1), g + 1, sg)
                stage_a_act(0, (0, 1), g, sg)
                if g == 0:
                    proj_q(1)
                elif 1 <= g <= 4:
                    proj_k(1, (g - 1,))
            sg = {}
            stage_a_qk(0, (2, 3), 0, sg)
            for g in range(8):
                if g < 7:
                    stage_a_qk(0, (2, 3), g + 1, sg)
                stage_a_act(0, (2, 3), g, sg)
                proj_v(2 * g)
                proj_v(2 * g + 1)

            po0 = [
                ps.tile([128, 512], F32, tag="po", bufs=2, name="po0a"),
                ps.tile([128, 512], F32, tag="po", bufs=2, name="po0b"),
            ]
            b0_sched = [[2 * i, 2 * i + 1] for i in range(8)]
            sg = {}
            stage_a_qk(1, (0, 1), 0, sg)
            for g in range(8):
                if g < 7:
                    stage_a_qk(1, (0, 1), g + 1, sg)
                stage_a_act(1, (0, 1), g, sg)
                stage_b_chunks(0, po0, b0_sched[g])
            stage_c_tile(0, 0, po0[0])
            stage_c_tile(0, 1, po0[1])

            po1 = [
                ps.tile([128, 512], F32, tag="po", bufs=2, name="po1a"),
                ps.tile([128, 512], F32, tag="po", bufs=2, name="po1b"),
            ]
            # quad-1 heads 4/5 (pv-tile a) were exp'd during A1-pair0, so
            # their PV runs early and po1a normalizes mid-pair1; heads 6/7
            # trail their own exps; only po1b's normalize sits in the tail.
            b1a_sched = [[0, 1, 2], [3, 4, 5], [6, 7], [8, 9], [10, 11], [12, 13], [14, 15], []]
            b1b_sched = [[], [0, 1], [2, 3], [4, 5], [6, 7], [8, 9], [10, 11], [12, 13, 14, 15]]
            sg = {}
            stage_a_qk(1, (2, 3), 0, sg)
            for g in range(8):
                if g < 7:
                    stage_a_qk(1, (2, 3), g + 1, sg)
                stage_a_act(1, (2, 3), g, sg)
                stage_b_chunks(1, po1, b1a_sched[g], rs=(0, 1))
                stage_b_chunks(1, po1, b1b_sched[g], rs=(2, 3))
                if g == 6:
                    stage_c_tile(1, 0, po1[0])
            stage_c_tile(1, 1, po1[1])

            # ---- final projection outT = Wo @ concatT + bo, as K=32
            # matmuls per (quad, pv-tile, half) against the permuted WoT.
            # accumulation groups must keep a constant tile row position
            # (mixing row bases 0/64 in one group faults the HW), so even-
            # and odd-head slices accumulate separately and DVE combines.
            out_sb = cst.tile([128, 2, QB], F32)
            for o in range(2):
                pf = {}
                for base in (0, 64):
                    pf[base] = ps.tile(
                        [128, QB], F32, tag="sc", bufs=3, name=f"pf{o}{base}"
                    )
                    idx = 0
                    for m in range(2):
                        for t in range(2):
                            nc.tensor.matmul(
                                pf[base][:],
                                wo_t[base : base + DH, 2 * m + t, ts(o, 128)],
                                prod[(m, t)][base : base + DH, :],
                                start=(idx == 0),
                                stop=(idx == 3),
                                tile_position=(base, 0),
                                skip_group_check=True,
                            )
                            idx += 1
                nc.vector.tensor_scalar_add(out_sb[:, o, :], pf[0][:], bo_t[:, o, :])
                nc.vector.tensor_add(out_sb[:, o, :], out_sb[:, o, :], pf[64][:])
                nc.sync.dma_start(outT[ts(o, 128), :], out_sb[:, o, :])

    nc.compile()
    return nc


_NC = None


def _get_nc():
    global _NC
    if _NC is None:
        _NC = _build_graph()
    return _NC


def prep_in_maps(query, key, value, Wq, Wk, Wv, Wo, bo):
    bf = ml_dtypes.bfloat16
    scale = np.float32(1.0 / np.sqrt(DH))

    wqT = np.ascontiguousarray((Wq.astype(np.float32) * scale).T).astype(bf)
    wkT = np.ascontiguousarray(Wk.T).astype(bf)
    # augmented WvT: [257 in-feats (last = ones row), 8 heads x 33 out-cols]
    wvT_a = np.zeros((D + 1, H * (DH + 1)), np.float32)
    wvt = Wv.T.astype(np.float32)  # [in 256, out 256]
    for h in range(H):
        wvT_a[:D, (DH + 1) * h : (DH + 1) * h + DH] = wvt[:, DH * h : DH * (h + 1)]
        wvT_a[D, (DH + 1) * h + DH] = 1.0
    wvT = np.ascontiguousarray(wvT_a).astype(bf)
    # permuted WoT: head h = 4m + 2t + (half==64) lives at partition rows
    # 64*(h%2) .. +32 of free-block 2m+t, matching PV output partitions.
    woP = np.zeros((128, 4, D), np.float32)
    woT = Wo.T.astype(np.float32)  # [hd, out]
    for h in range(H):
        m, r = h // 4, h % 4
        blk, base = 2 * m + r // 2, 64 * (r % 2)
        woP[base : base + DH, blk, :] = woT[DH * h : DH * (h + 1), :]
    woP = np.ascontiguousarray(woP.reshape(128, 4 * D)).astype(bf)
    bo_c = np.ascontiguousarray(bo.astype(np.float32).reshape(D, 1))

    kT_b = [np.ascontiguousarray(key[b].T).astype(bf) for b in range(B)]
    vT_b = [np.ascontiguousarray(value[b].T).astype(bf) for b in range(B)]

    in_maps = []
    for c in range(8):
        b, qb = c // 4, c % 4
        in_maps.append(
            {
                "qT": np.ascontiguousarray(
                    query[b, qb * QB : (qb + 1) * QB, :].T
                ).astype(bf),
                "kT": kT_b[b],
                "vT": vT_b[b],
                "wqT": wqT,
                "wkT": wkT,
                "wvT": wvT,
                "woP": woP,
                "bo": bo_c,
            }
        )
    return in_maps


def kernel(query, key, value, Wq, Wk, Wv, Wo, bo):
    nc = _get_nc()
    in_maps = prep_in_maps(query, key, value, Wq, Wk, Wv, Wo, bo)
    res = run_bass_kernel_spmd(nc, in_maps, core_ids=list(range(8)))

    out = np.empty((B, SEQ, D), np.float32)
    for c in range(8):
        b, qb = c // 4, c % 4
        out[b, qb * QB : (qb + 1) * QB, :] = res.results[c]["outT"].T
    return out

